# revision 8
# baseline (speedup 1.0000x reference)
"""Causal single-head attention (B=4, S=4096, D=768) on 8 TRN2 NeuronCores.

Sharding: core = (batch b = core//2, half h = core%2), causally-balanced
query interleave as in the fp16 baseline (see kernel.py docstring).

Every large matmul runs in fp8(e4m3) DoubleRow perf mode (256-deep
contraction at 0.5 PE cycles per output column = 4x the fp16 rate) with
two-level (hi+lo) operand compensation: each fp16 tensor T is stored as
Th = fp8(T), Tl = fp8(T - Th), and products use the 3-sweep expansion
Ah.Bh + Al.Bh + Ah.Bl (dropping the ~2^-8 Al.Bl term): 4.5 output-columns
per 128-contraction vs fp16's 6.0 -> 1.33x at ~2^-8 operand precision.

Subnormal-avoidance scaling: fp8 e4m3 normals start at 2^-6, so operands
are pre-scaled into range: W is uploaded x16 (entries ~0.58), making the
projection PSUM 16q; q/k are stored from that PSUM as-is (scores PSUM is
then 256 q.k, absorbed into the exp scale), and exp uses bias -1.5 so
P = e^(z-1.5) <= ~160 < 240 with its mass in normal range. Softmax is
shift-invariant and l is accumulated from the same Ph+Pl tiles, so the
bias and all scales cancel exactly.

Each two-level tensor keeps hi/lo in one tile with a `level` dim (one DMA
moves both); DoubleRow pairs slice adjacent d-tiles (or key tiles) at a
fixed level. P-pair tiles [P, level, slot, QG] hold key tiles (2p, 2p+1)
in the two pair slots, feeding the x^T P accumulation directly. The Ut
matmuls for pair p are issued after the scores of pair p+1 so the PE
stays busy while the exp->copy->sub extraction chain of pair p drains.
ctx = (P@x)@Wv with an f32r final GEMM, denominator via fp8 ones-matmuls.
"""

import math

import numpy as np
import ml_dtypes

B, S, D = 4, 4096, 768
P = 128
DT = D // P            # 6 d-tiles
NK = S // P            # 32 key tiles
NG = 4                 # query groups per core
QG = 512               # query columns per group
NSLOT = 16             # 128-row query blocks per core
QW = NSLOT * P         # 2048 query rows per core
WS = 16.0              # W pre-scale (q,k stored x16)
SCALE_Z = (1.0 / math.sqrt(D)) / (WS * WS)
CBIAS = 1.5            # exp bias: P = e^(z - CBIAS)

F16 = np.float16
F8 = ml_dtypes.float8_e4m3

_CACHE = {}


def _build():
    import concourse.tile as tile
    from concourse import bacc, mybir

    f32 = mybir.dt.float32
    f32r = mybir.dt.float32r
    f16 = mybir.dt.float16
    f8 = mybir.dt.float8e4
    Exp = mybir.ActivationFunctionType.Exp
    Copy = mybir.ActivationFunctionType.Copy
    DR = mybir.MatmulPerfMode.DoubleRow

    nc = bacc.Bacc(
        "TRN2",
        target_bir_lowering=False,
        debug=False,
        enable_asserts=False,
        num_devices=8,
    )

    xtp = nc.dram_tensor("xtp", [P, DT, 2, S], f8, kind="ExternalInput").ap()
    xqp = nc.dram_tensor("xqp", [P, DT, 2, QW], f8, kind="ExternalInput").ap()
    xnp = nc.dram_tensor("xnp", [P, NK, 2, D], f8, kind="ExternalInput").ap()
    wqp = nc.dram_tensor("wqp", [P, DT, 2, D], f8, kind="ExternalInput").ap()
    wkp = nc.dram_tensor("wkp", [P, DT, 2, D], f8, kind="ExternalInput").ap()
    wv = nc.dram_tensor("wv", [P, DT, D], f16, kind="ExternalInput").ap()
    masks = nc.dram_tensor("masks", [2, P, P], f16, kind="ExternalInput").ap()
    out = nc.dram_tensor("out", [QW, D], f16, kind="ExternalOutput").ap()

    with tile.TileContext(nc, pool_alloc_mode="queue") as tc:
        pool_eng = nc.engines[mybir.EngineType.Pool]
        with tc.tile_pool(name="resid", bufs=1) as resid:
            ktp = resid.tile([P, DT, 2, S], f8, tag="ktp")
            qtp = resid.tile([P, DT, 2, QW], f8, tag="qtp")
            xn_sb = resid.tile([P, NK, 2, D], f8, tag="xn")
            wv_r = resid.tile([P, DT, D], mybir.dt.float32r, tag="wvr")
            mask_sb = resid.tile([P, 2, P], f16, tag="mask")
            ones8 = resid.tile([P, 2, 1], f8, tag="ones8")
            bias_c = resid.tile([P, 1], f32, tag="biasc")

            # ---------------- Phase 1: projections ----------------
            with (
                tc.tile_pool(name="wp", bufs=1) as wp,
                tc.tile_pool(name="xin", bufs=3) as xin,
                tc.tile_pool(name="psP", bufs=4, space="PSUM") as psP,
            ):
                wq_sb = wp.tile([P, DT, 2, D], f8, tag="wq")
                wk_sb = wp.tile([P, DT, 2, D], f8, tag="wk")
                wv_sb = wp.tile([P, DT, D], f16, tag="wv16")

                # Q projection: 4 chunks of 512 queries
                for qc in range(QW // 512):
                    xch = xin.tile([P, DT, 2, 512], f8, tag="xin")
                    if qc == 0:
                        # pair-aligned thirds, first-needed first in the
                        # HWDGE queue (the do-chain consumes di pairs in
                        # order 01, 23, 45)
                        for dp in range(3):
                            nc.sync.dma_start(wq_sb[:, 2 * dp:2 * dp + 2],
                                              wqp[:, 2 * dp:2 * dp + 2])
                            nc.sync.dma_start(
                                xch[:, 2 * dp:2 * dp + 2],
                                xqp[:, 2 * dp:2 * dp + 2, :, 0:512])
                        nc.vector.memset(bias_c[:], -CBIAS)
                        nc.vector.memset(ones8[:], 1.0)
                    else:
                        nc.sync.dma_start(
                            xch[:],
                            xqp[:, :, :, qc * 512:(qc + 1) * 512])
                    if qc == 1:
                        nc.sync.dma_start(wk_sb[:], wkp[:])
                    if qc == 2:
                        nc.sync.dma_start(wv_sb[:], wv[:])
                        nc.vector.tensor_copy(wv_r[:], wv_sb[:])
                        for r in range(2):
                            nc.sync.dma_start(mask_sb[:, r, :], masks[r, :, :])
                    cols = slice(qc * 512, (qc + 1) * 512)
                    for do in range(DT):
                        ps = psP.tile([P, 512], f32)
                        n = 0
                        for wl, xl_ in ((0, 0), (1, 0), (0, 1)):
                            for dp in range(3):
                                nc.tensor.matmul(
                                    ps[:],
                                    wq_sb[:, 2 * dp:2 * dp + 2, wl,
                                          do * P:(do + 1) * P],
                                    xch[:, 2 * dp:2 * dp + 2, xl_, :],
                                    start=(n == 0),
                                    stop=(n == 8),
                                    perf_mode=DR,
                                )
                                n += 1
                        nc.scalar.activation(qtp[:, do, 0, cols], ps[:], Copy)
                        nc.vector.tensor_sub(qtp[:, do, 1, cols], ps[:],
                                             qtp[:, do, 0, cols])

                # K projection: 8 chunks of 512 keys; xn loaded in 2 halves
                for kc in range(S // 512):
                    xch = xin.tile([P, DT, 2, 512], f8, tag="xin")
                    nc.sync.dma_start(
                        xch[:],
                        xtp[:, :, :, kc * 512:(kc + 1) * 512])
                    if kc == 0:
                        nc.sync.dma_start(xn_sb[:, 0:NK // 2],
                                          xnp[:, 0:NK // 2])
                    if kc == 2:
                        nc.sync.dma_start(xn_sb[:, NK // 2:NK],
                                          xnp[:, NK // 2:NK])
                    cols = slice(kc * 512, (kc + 1) * 512)
                    for do in range(DT):
                        ps = psP.tile([P, 512], f32)
                        n = 0
                        for wl, xl_ in ((0, 0), (1, 0), (0, 1)):
                            for dp in range(3):
                                nc.tensor.matmul(
                                    ps[:],
                                    wk_sb[:, 2 * dp:2 * dp + 2, wl,
                                          do * P:(do + 1) * P],
                                    xch[:, 2 * dp:2 * dp + 2, xl_, :],
                                    start=(n == 0),
                                    stop=(n == 8),
                                    perf_mode=DR,
                                )
                                n += 1
                        nc.scalar.activation(ktp[:, do, 0, cols], ps[:], Copy)
                        nc.vector.tensor_sub(ktp[:, do, 1, cols], ps[:],
                                             ktp[:, do, 0, cols])

            # ------------- Phase 2: attention -------------
            with (
                tc.tile_pool(name="ptp", bufs=18) as ptp,
                tc.tile_pool(name="pmp", bufs=6) as pmp,
                tc.tile_pool(name="utsb", bufs=8) as utsb,
                tc.tile_pool(name="outp", bufs=2) as outp,
                tc.tile_pool(name="small", bufs=4) as small,
                tc.tile_pool(name="psS", bufs=2, space="PSUM") as psS,
                tc.tile_pool(name="utp", bufs=6, space="PSUM") as utp,
            ):
                for t in range(NG):
                    win = 8 * t + 8
                    npair = win // 2
                    pts = []
                    c0s = []
                    ut_ps = [utp.tile([P, QG], f32, tag="ut",
                                      name=f"ut_{t}_{i}")
                             for i in range(6)]

                    def emit_ut(p):
                        c0 = c0s[p]
                        pt2 = pts[p]
                        n = 0
                        for xl_, pl_ in ((0, 0), (0, 1), (1, 0)):
                            for di in range(6):
                                nc.tensor.matmul(
                                    ut_ps[di][:, c0:QG],
                                    xn_sb[:, 2 * p:2 * p + 2, xl_,
                                          di * P:(di + 1) * P],
                                    pt2[:, pl_, :, c0:QG],
                                    start=(p == 0 and n // 6 == 0),
                                    stop=(p == npair - 1 and n // 6 == 2),
                                    perf_mode=DR,
                                )
                                n += 1

                    for p in range(npair):
                        c0 = (p - 4 * t) * P if p - 4 * t >= 1 else 0
                        c0s.append(c0)
                        pt2 = ptp.tile([P, 2, 2, QG], f8, tag="pt")
                        for e in range(2):
                            k = 2 * p + e
                            ps = psS.tile([P, QG], f32)
                            n = 0
                            for kl_, ql_ in ((0, 0), (1, 0), (0, 1)):
                                for dp in range(3):
                                    nc.tensor.matmul(
                                        ps[:, c0:QG],
                                        ktp[:, 2 * dp:2 * dp + 2, kl_,
                                            k * P:(k + 1) * P],
                                        qtp[:, 2 * dp:2 * dp + 2, ql_,
                                            t * QG + c0:(t + 1) * QG],
                                        start=(n == 0),
                                        stop=(n == 8),
                                        perf_mode=DR,
                                    )
                                    n += 1
                            pm = pmp.tile([P, QG], f16, tag="pm")
                            nc.scalar.activation(pm[:, c0:QG], ps[:, c0:QG],
                                                 Exp, scale=SCALE_Z,
                                                 bias=bias_c[:])
                            if k >= 8 * t:
                                j = (k - 8 * t) // 2
                                rel = (k - 8 * t) % 2
                                nc.vector.tensor_mul(
                                    pm[:, j * P:(j + 1) * P],
                                    pm[:, j * P:(j + 1) * P],
                                    mask_sb[:, rel, :],
                                )
                            pool_eng.tensor_copy(pt2[:, 0, e, c0:QG],
                                                 pm[:, c0:QG])
                            nc.vector.tensor_sub(pt2[:, 1, e, c0:QG],
                                                 pm[:, c0:QG],
                                                 pt2[:, 0, e, c0:QG])
                        pts.append(pt2)
                        if p >= 2:
                            emit_ut(p - 2)
                    emit_ut(npair - 2)
                    emit_ut(npair - 1)

                    ut_sb = []
                    for di in range(6):
                        u = utsb.tile([P, QG], f32r, tag="ut_sb",
                                      name=f"utsb_{t}_{di}")
                        nc.vector.tensor_copy(u[:], ut_ps[di][:])
                        ut_sb.append(u)

                    for j in range(4):
                        pso = utp.tile([P, 512], f32, tag="ut")
                        pso2f = utp.tile([P, 512], f32, tag="ut")
                        pso2 = pso2f[:, 0:256]
                        for di in range(DT):
                            nc.tensor.matmul(
                                pso[:],
                                ut_sb[di][:, j * P:(j + 1) * P],
                                wv_r[:, di, 0:512],
                                start=(di == 0),
                                stop=(di == DT - 1),
                            )
                        for di in range(DT):
                            nc.tensor.matmul(
                                pso2[:],
                                ut_sb[di][:, j * P:(j + 1) * P],
                                wv_r[:, di, 512:768],
                                start=(di == 0),
                                stop=(di == DT - 1),
                            )
                        npj = 4 * t + j + 1
                        pslf = utp.tile([P, 512], f32, tag="ut")
                        psl = pslf[:, 0:1]
                        n = 0
                        for p in range(npj):
                            for lvl in range(2):
                                nc.tensor.matmul(
                                    psl[:],
                                    pts[p][:, lvl, :, j * P:(j + 1) * P],
                                    ones8[:, :, 0:1],
                                    start=(n == 0),
                                    stop=(n == 2 * npj - 1),
                                    perf_mode=DR,
                                )
                                n += 1
                        linv = small.tile([P, 1], f32, tag="linv")
                        nc.vector.reciprocal(linv[:], psl[:])
                        osb = outp.tile([P, D], f16, tag="osb")
                        nc.vector.tensor_scalar_mul(osb[:, 0:512],
                                                    pso[:], linv[:])
                        nc.vector.tensor_scalar_mul(osb[:, 512:768],
                                                    pso2[:], linv[:])
                        s = 4 * t + j
                        nc.sync.dma_start(out[s * P:(s + 1) * P, :], osb[:])

    nc.compile()
    return nc


def _get_nc():
    if "nc" not in _CACHE:
        _CACHE["nc"] = _build()
    return _CACHE["nc"]


def _split8(a32):
    hi = a32.astype(F8)
    lo = (a32 - hi.astype(np.float32)).astype(F8)
    return hi, lo


def _make_in_maps(x, Wq, Wk, Wv):
    x = np.asarray(x, dtype=np.float32)
    wq16 = np.asarray(Wq, dtype=np.float32).astype(F16).astype(np.float32)
    wk16 = np.asarray(Wk, dtype=np.float32).astype(F16).astype(np.float32)
    wv16 = np.asarray(Wv, dtype=np.float32).astype(F16)

    def _pout(a):
        # [D, D'] -> partition-outer [P, DT, D']
        return a.reshape(DT, P, -1).transpose(1, 0, 2)

    wqh, wql = _split8(wq16 * WS)
    wkh, wkl = _split8(wk16 * WS)
    # [P, DT, 2, D]
    wqp = np.ascontiguousarray(np.stack([_pout(wqh), _pout(wql)], axis=2))
    wkp = np.ascontiguousarray(np.stack([_pout(wkh), _pout(wkl)], axis=2))
    wv_po = np.ascontiguousarray(_pout(wv16))  # [P, DT, D]

    tri = (np.arange(P)[:, None] <= np.arange(P)[None, :]).astype(np.float32)
    ones = np.ones((P, P), dtype=np.float32)
    zeros = np.zeros((P, P), dtype=np.float32)
    mask_h = [
        np.stack([tri, zeros]).astype(F16),  # h=0: rel0 tri, rel1 zero
        np.stack([ones, tri]).astype(F16),   # h=1: rel0 ones, rel1 tri
    ]

    xsh = np.ascontiguousarray(x.astype(F16).reshape(8 * QW, D))
    in_maps = []
    for core in range(8):
        h = core % 2
        in_maps.append(
            {
                "xsh": xsh,
                "wqp": wqp,
                "wkp": wkp,
                "wv": wv_po,
                "masks": mask_h[h],
            }
        )
    return in_maps


def _get_exec():
    """Build (once) a cached jitted SPMD callable over 8 cores."""
    if "exec" in _CACHE:
        return _CACHE["exec"]

    import jax
    from jax.sharding import Mesh, PartitionSpec
    from jax.experimental.shard_map import shard_map
    import concourse.mybir as mybir
    from concourse.bass2jax import (
        _bass_exec_p,
        install_neuronx_cc_hook,
        partition_id_tensor,
    )

    install_neuronx_cc_hook()
    nc = _get_nc()
    partition_name = nc.partition_id_tensor.name if nc.partition_id_tensor else None

    in_names, out_names, out_avals, zero_shapes = [], [], [], []
    for alloc in nc.m.functions[0].allocations:
        if not isinstance(alloc, mybir.MemoryLocationSet):
            continue
        name = alloc.memorylocations[0].name
        if alloc.kind == "ExternalInput":
            if name == partition_name:
                continue
            in_names.append(name)
        elif alloc.kind == "ExternalOutput":
            out_names.append(name)
            shape = tuple(alloc.tensor_shape)
            dtype = mybir.dt.np(alloc.dtype)
            out_avals.append(jax.core.ShapedArray(shape, dtype))
            zero_shapes.append((shape, dtype))
    n_params = len(in_names)
    n_outs = len(out_avals)
    all_names = in_names + out_names
    if partition_name is not None:
        all_names = all_names + [partition_name]
    donate = tuple(range(n_params, n_params + n_outs))

    def _body(*args):
        operands = list(args)
        if partition_name is not None:
            operands.append(partition_id_tensor())
        outs = _bass_exec_p.bind(
            *operands,
            out_avals=tuple(out_avals),
            in_names=tuple(all_names),
            out_names=tuple(out_names),
            lowering_input_output_aliases=(),
            sim_require_finite=True,
            sim_require_nnan=True,
            nc=nc,
        )
        return tuple(outs)

    devices = jax.devices()[:8]
    mesh = Mesh(np.asarray(devices), ("core",))
    replicated = {"wqp", "wkp", "wv"}
    in_specs = tuple(
        PartitionSpec() if name in replicated else PartitionSpec("core")
        for name in in_names
    ) + (PartitionSpec("core"),) * n_outs
    sharded = jax.jit(
        shard_map(
            _body,
            mesh=mesh,
            in_specs=in_specs,
            out_specs=(PartitionSpec("core"),) * n_outs,
            check_rep=False,
        ),
        donate_argnums=donate,
        keep_unused=True,
    )

    # On-device input prep: all_gather the batch, transpose in fp16, then
    # split into stacked fp8 hi/lo pairs for all three layouts. Two jit
    # stages with the hi casts materialized to HBM between them: inside a
    # single jit the neuron XLA pipeline fuses convert(convert(a,f8),f32)
    # into identity, which silently turns every lo into zero.
    def _prep_a(x_shard):
        import jax.numpy as jnp
        from jax import lax

        f8 = jnp.float8_e4m3

        h = lax.axis_index("core") % 2
        x_full = lax.all_gather(
            x_shard,
            "core",
            axis_index_groups=[[0, 1], [2, 3], [4, 5], [6, 7]],
            axis=0,
            tiled=True,
        )  # [S, D] f16
        # partition-outer layouts (transposes done in f16)
        xt = jnp.transpose(x_full).reshape(DT, P, S).transpose(1, 0, 2)
        xqrows = lax.dynamic_slice_in_dim(
            x_full.reshape(NSLOT, 2, P, D), h, 1, axis=1
        ).reshape(QW, D)
        xq = jnp.transpose(xqrows).reshape(DT, P, QW).transpose(1, 0, 2)
        xn = x_full.reshape(NK, P, D).transpose(1, 0, 2)  # [P, NK, D]
        return (xt, xq, xn,
                xt.astype(f8), xq.astype(f8), xn.astype(f8))

    def _prep_b(xt, xq, xn, xth, xqh, xnh):
        import jax.numpy as jnp

        f8 = jnp.float8_e4m3

        def pair(a16, hi8):
            lo = (a16.astype(jnp.float32)
                  - hi8.astype(jnp.float32)).astype(f8)
            return jnp.stack([hi8, lo], axis=2)

        return pair(xt, xth), pair(xq, xqh), pair(xn, xnh)

    prep_a = jax.jit(
        shard_map(
            _prep_a,
            mesh=mesh,
            in_specs=(PartitionSpec("core"),),
            out_specs=(PartitionSpec("core"),) * 6,
            check_rep=False,
        )
    )
    prep_b = jax.jit(
        shard_map(
            _prep_b,
            mesh=mesh,
            in_specs=(PartitionSpec("core"),) * 6,
            out_specs=(PartitionSpec("core"),) * 3,
            check_rep=False,
        )
    )

    def prep(xsh):
        import jax as _jax
        mids = prep_a(xsh)
        _jax.block_until_ready(mids)
        return prep_b(*mids)
    _CACHE["exec"] = (
        sharded, in_names, out_names, out_avals, zero_shapes, replicated, prep, mesh,
    )
    return _CACHE["exec"]


def _concat_inputs(in_maps, in_names, replicated):
    return [
        np.asarray(in_maps[0][name])
        if name in replicated
        else np.concatenate([np.asarray(m[name]) for m in in_maps], axis=0)
        for name in in_names
    ]


def _make_zeros(zero_shapes):
    return [
        np.zeros((8 * shape[0], *shape[1:]), dtype) for shape, dtype in zero_shapes
    ]


def _prep_staged(in_maps):
    (sharded, in_names, out_names, out_avals, zero_shapes, replicated,
     prep, mesh) = _get_exec()
    xtp, xqp, xnp_ = prep(in_maps[0]["xsh"])
    return {"xtp": xtp, "xqp": xqp, "xnp": xnp_}


def _run(in_maps):
    import jax

    (sharded, in_names, out_names, out_avals, zero_shapes, replicated,
     prep, mesh) = _get_exec()
    staged = _prep_staged(in_maps)
    concat_in = [
        staged[name] if name in staged
        else _concat_inputs(in_maps, [name], replicated)[0]
        for name in in_names
    ]
    donated = _CACHE.pop("outbuf", None)
    if donated is None:
        donated = _make_zeros(zero_shapes)
    out_arrs = sharded(*concat_in, *donated)
    _CACHE["outbuf"] = list(out_arrs)
    i = out_names.index("out")
    full = np.asarray(out_arrs[i]).reshape(8, *out_avals[i].shape)
    return [full[c] for c in range(8)]


def kernel(x, Wq, Wk, Wv):
    in_maps = _make_in_maps(x, Wq, Wk, Wv)
    outs = _run(in_maps)
    out = np.empty((B, S, D), dtype=np.float32)
    for core in range(8):
        b, h = core // 2, core % 2
        out[b].reshape(NSLOT, 2, P, D)[:, h] = outs[core].reshape(NSLOT, P, D)
    return out
